# revision 1
# baseline (speedup 1.0000x reference)
"""ClusterGCN layer on 8 TRN2 NeuronCores.

Math: for each cluster c (only intra-cluster edges matter),
    Y_c = B_c @ (X_c @ W) + b
    B_c[d, s] = dis[d] * At_c[s, d] * dis[s]
    At_c[s, d] = #edges(s->d in c) + [s == d]     (self-loop: dis^2 = 1/deg)
with dis = rsqrt(deg), deg = intra in-degree + 1. Clusters with no intra
edge pass X through unchanged (patched on host).

Device per core (pipelined over clusters), all matmuls fp16 on the PE
(fp16 matches bf16 PE throughput with 4x the mantissa; values are O(1)):
  step1: xws = (X @ W) * dis[s]   -- nodes on partitions; the dis scale is
         folded into the PSUM->SBUF cast on the Scalar engine.
  step2: Z_c^T = xws^T-chunks (stationary) x At_c (moving, N=cap), i.e.
         Z^T[f, d] = sum_s xws[s,f] * At[s,d];  Y_c = dis[d] * Z[d] (host).
At ships as fp8e4m3 edge counts (integer counts <= 16 are exact in
e4m3, and the PE accepts a mixed fp16-stationary x fp8-moving matmul).
Output is per-cluster transposed [f, cap]; host de-transposes in the
gather, applies dis[d] and the bias.
"""

import os

import numpy as np

N_CORES = 8
N_CLUSTERS = 100
P = 128

# compute dtype for X/W/xW tiles: fp16 matches bf16 PE throughput with 4x
# the mantissa (all values here are O(1), so fp16 range is safe)
_X_DT = os.environ.get("KX_DTYPE", "fp16")
_Y_DT = os.environ.get("KYT_DTYPE", "fp16")

_prog_cache: dict = {}


def _build_program(cpc: int, cap: int, in_c: int, f_out: int, fp8_path: bool):
    """Build + compile the per-core Bass program.

    cpc: clusters per core; cap: padded cluster size (multiple of 128).
    fp8_path: adjacency as fp8e4m3 counts + on-device scaling (default);
    the bf16 fallback ships pre-scaled B^T blocks (counts > 16 only).
    """
    import concourse.mybir as mybir
    import concourse.tile as tile
    from concourse import bacc

    key = (cpc, cap, in_c, f_out, fp8_path, _X_DT, _Y_DT)
    if key in _prog_cache:
        return _prog_cache[key]

    kc = in_c // P           # contraction chunks for X @ W
    sch = cap // P           # s chunks per cluster
    fc = f_out // P          # f chunks (step-2 output partitions)
    f32 = mybir.dt.float32
    f32r = mybir.dt.float32r
    dt_of = {"bf16": mybir.dt.bfloat16, "fp16": mybir.dt.float16,
             "f32": f32, "f32r": f32r}
    fp8 = mybir.dt.float8e4
    x_dt = dt_of[_X_DT]
    a_dt = fp8 if fp8_path else x_dt

    XG = 4                   # clusters per XT load group
    AG = 2                   # clusters per At load group

    nc = bacc.Bacc("TRN2", target_bir_lowering=False, debug=False,
                   num_devices=N_CORES)

    XT = nc.dram_tensor("XT", [in_c, cpc * cap], x_dt, kind="ExternalInput")
    Wt = nc.dram_tensor("Wt", [in_c, f_out], x_dt, kind="ExternalInput")
    AT = nc.dram_tensor("AT", [cpc, P, sch, cap], a_dt, kind="ExternalInput")
    DIS = nc.dram_tensor("DIS", [P, cpc, sch], f32, kind="ExternalInput")
    y_dt = dt_of[_Y_DT]
    YT = nc.dram_tensor("YT", [cpc, f_out, cap], y_dt, kind="ExternalOutput")

    XTr = XT.rearrange("(k p) n -> p k n", p=P)

    with tile.TileContext(nc) as tc:
        with (
            tc.tile_pool(name="w", bufs=1) as w_pool,
            tc.tile_pool(name="xt", bufs=3) as xt_pool,
            tc.tile_pool(name="at", bufs=4) as at_pool,
            tc.tile_pool(name="xw", bufs=4 * sch) as xw_pool,
            tc.tile_pool(name="out", bufs=5) as out_pool,
            tc.tile_pool(name="ps1", bufs=4, space="PSUM") as ps1_pool,
            tc.tile_pool(name="ps2", bufs=4, space="PSUM") as ps2_pool,
        ):
            # small loads go on the Scalar queue so the Sync queue's first
            # work is cluster 0's data
            wt = w_pool.tile([P, kc, f_out], x_dt)
            nc.scalar.dma_start(wt[:], Wt.rearrange("(k p) f -> p k f", p=P))
            dcol = w_pool.tile([P, cpc, sch], f32)
            nc.scalar.dma_start(dcol[:], DIS[:])


            # group sizes: a small first load so compute starts early
            def groups(g):
                sizes, c0 = [], 0
                first = True
                while c0 < cpc:
                    g_ = 1 if first else min(g, cpc - c0)
                    sizes.append((c0, min(g_, cpc - c0)))
                    c0 += sizes[-1][1]
                    first = False
                return sizes

            xg_of = {}
            for c0, g in groups(XG):
                for c in range(c0, c0 + g):
                    xg_of[c] = (c0, g)
            ag_of = {}
            for c0, g in groups(AG):
                for c in range(c0, c0 + g):
                    ag_of[c] = (c0, g)

            xt = at = None
            for c in range(cpc):
                c0, g = xg_of[c]
                if c == c0:
                    xt = xt_pool.tile([P, kc, XG * cap], x_dt)
                    nc.sync.dma_start(
                        xt[:, :, :g * cap],
                        XTr[:, :, c0 * cap:(c0 + g) * cap],
                    )
                a0, ag = ag_of[c]
                if c == a0:
                    at = at_pool.tile([P, AG, sch, cap], a_dt)
                    nc.sync.dma_start(
                        at[:, :ag],
                        AT[a0:a0 + ag].rearrange("c p so d -> p c so d"),
                    )
                xoff = (c - c0) * cap
                ci = c - a0

                ab = at[:, ci]

                # step 1: xws = (X @ W) * dis, nodes on partitions
                xw_tiles = []
                for t in range(sch):
                    ps = ps1_pool.tile([P, f_out], f32)
                    for k in range(kc):
                        nc.tensor.matmul(
                            ps[:],
                            lhsT=xt[:, k, xoff + t * P:xoff + (t + 1) * P],
                            rhs=wt[:, k, :],
                            start=(k == 0),
                            stop=(k == kc - 1),
                        )
                    xw = xw_pool.tile([P, f_out], x_dt)
                    if fp8_path:
                        nc.scalar.activation(
                            xw[:], ps[:], mybir.ActivationFunctionType.Copy,
                            scale=dcol[:, c, t:t + 1],
                        )
                    else:
                        nc.scalar.copy(xw[:], ps[:])
                    xw_tiles.append(xw)

                # step 2: Z_c^T = sum_s xws-chunk^T x At rows, f on
                # partitions; d chunked to the 512-fp32 PSUM bank limit
                ot = out_pool.tile([P, fc, cap], y_dt)
                for f in range(fc):
                    for d0 in range(0, cap, 512):
                        dn = min(512, cap - d0)
                        ps = ps2_pool.tile([P, 512], f32)
                        for s in range(sch):
                            nc.tensor.matmul(
                                ps[:, :dn],
                                lhsT=xw_tiles[s][:, f * P:(f + 1) * P],
                                rhs=ab[:, s, d0:d0 + dn],
                                start=(s == 0),
                                stop=(s == sch - 1),
                            )
                        nc.vector.tensor_copy(
                            ot[:, f, d0:d0 + dn], ps[:, :dn])
                nc.sync.dma_start(
                    YT[c].rearrange("(f p) d -> p f d", p=P), ot[:]
                )

    nc.compile()
    _prog_cache[key] = nc
    return nc


def _host_prep(X, W, b, assign, full_ei):
    """Shard + preprocess. Returns (in_maps, fp8_path, gather info)."""
    n, in_c = X.shape
    f_out = W.shape[1]
    src = full_ei[0].astype(np.int64)
    dst = full_ei[1].astype(np.int64)
    a_s = assign[src]
    intra = a_s == assign[dst]
    es, ed = src[intra], dst[intra]

    deg = np.ones(n, np.float32)
    np.add.at(deg, ed, np.float32(1))
    dis = (1.0 / np.sqrt(deg)).astype(np.float32)

    has_edge = np.zeros(N_CLUSTERS, bool)
    has_edge[np.unique(a_s[intra])] = True

    sizes = np.bincount(assign, minlength=N_CLUSTERS)
    cpc = -(-N_CLUSTERS // N_CORES)            # clusters per core
    cap = max(512, int(-(-sizes.max() // P)) * P)  # padded cluster size

    starts = np.zeros(N_CLUSTERS + 1, np.int64)
    starts[1:] = np.cumsum(sizes)
    order = np.argsort(assign, kind="stable")
    pos = np.empty(n, np.int64)
    pos[order] = np.arange(n) - starts[assign[order]]

    ctot = cpc * N_CORES
    # At blocks: At[c][s, d] = #edges(s->d) + [s==d]
    At = np.zeros((ctot, cap, cap), np.uint16)
    np.add.at(At, (assign[es], pos[es], pos[ed]), 1)
    At[assign, pos, pos] += 1
    fp8_path = int(At.max()) <= 16    # integers <= 16 are exact in e4m3

    import ml_dtypes
    x_np = {"bf16": ml_dtypes.bfloat16, "fp16": np.float16,
            "f32": np.float32, "f32r": np.float32}[_X_DT]

    Xp = np.zeros((ctot, cap, in_c), np.float32)
    Xp[assign, pos] = X
    XT_all = np.ascontiguousarray(Xp.reshape(ctot * cap, in_c).T)

    DISp = np.zeros((ctot, cap), np.float32)
    DISp[assign, pos] = dis

    if fp8_path:
        import concourse.mybir as mybir
        At_send = At.astype(mybir.dt.np(mybir.dt.float8e4))
    else:
        # rare fallback: pre-scaled B^T blocks in the compute dtype
        At_send = (At.astype(np.float32)
                   * DISp[:, :, None] * DISp[:, None, :]).astype(x_np)
    # [c, s, d] -> [c, p, so, d] so each partition row is one 2KB run
    sch = cap // P
    At_send = np.ascontiguousarray(
        At_send.reshape(-1, sch, P, cap).transpose(0, 2, 1, 3))
    DIS_send = np.ascontiguousarray(
        DISp.reshape(-1, sch, P).transpose(2, 0, 1))  # [128, ctot, sch]

    nodes = cpc * cap
    in_maps = []
    for i in range(N_CORES):
        in_maps.append({
            "XT": np.ascontiguousarray(
                XT_all[:, i * nodes:(i + 1) * nodes]).astype(x_np),
            "Wt": W.astype(np.float32).astype(x_np),
            "AT": At_send[i * cpc:(i + 1) * cpc],
            "DIS": np.ascontiguousarray(
                DIS_send[:, i * cpc:(i + 1) * cpc]),
        })
    return in_maps, fp8_path, (cpc, cap, has_edge, pos, dis)


def _run(inputs, trace=False, tmpdir=None):
    from concourse.bass_utils import run_bass_kernel_spmd

    X = np.asarray(inputs["X"], np.float32)
    W = np.asarray(inputs["W"], np.float32)
    b = np.asarray(inputs["b"], np.float32)
    assign = np.asarray(inputs["assign"])
    full_ei = np.asarray(inputs["full_ei"])

    n, in_c = X.shape
    f_out = W.shape[1]
    in_maps, fp8_path, (cpc, cap, has_edge, pos, dis) = _host_prep(
        X, W, b, assign, full_ei)
    nc = _build_program(cpc, cap, in_c, f_out, fp8_path)

    res = run_bass_kernel_spmd(
        nc, in_maps, core_ids=list(range(N_CORES)),
        trace=trace, tmpdir=tmpdir,
    )
    # YT: [core][cpc, f_out, cap]; row n lives at [core, lc, :, pos]
    YTdev = np.stack([res.results[i]["YT"] for i in range(N_CORES)])
    if YTdev.dtype != np.float32:
        YTdev = YTdev.astype(np.float32)

    c = assign.astype(np.int64)
    core = c // cpc
    lc = c % cpc
    Y = YTdev[core, lc, :, pos]
    if fp8_path:
        Y *= dis[:, None]
    Y += b[None, :].astype(np.float32)
    miss = ~has_edge[c]
    if miss.any():
        Y[miss] = X[miss]
    return Y, res


def kernel(**inputs) -> np.ndarray:
    Y, _ = _run(inputs)
    return Y



# revision 6
# speedup vs baseline: 1.0159x; 1.0159x over previous
"""ClusterGCN layer on 8 TRN2 NeuronCores.

Math per cluster c (only intra-cluster edges matter):
    Y_c = B_c @ X_c @ W + b,   B_c[d, s] = dis[d] * At_c[s, d] * dis[s]
    At_c[s, d] = #edges(s->d in c) + [s == d],  dis = rsqrt(deg)
Clusters with no intra edge pass X through (patched on host).

Device formulation (At-first, contraction always on the partition dim):
  step1: T^T[k, d] = sum_s Xs[s, k] * At[s, d]   per cluster
         lhsT = Xs chunks (stationary, [s:128, k:128]),
         rhs  = At s-chunk rows ([s:128, d:512], fp8 counts — integers
         <= 16 are exact in e4m3).  Xs = X * dis[s] (folded on host).
  step2: Y^T[f, d] = sum_k W[k, f] * T^T[k, d]
         lhsT = W chunks (stationary, only 4 distinct), rhs = T^T from
         SBUF.  Host applies dis[d] and + b after the gather.
All matmuls fp16xfp8 / fp16xfp16 (fp16 matches bf16 PE rate, 4x the
mantissa; values are O(1)).

A junk-data warm-up matmul burst runs at kernel start with no data
dependencies: it spans the initial DMA latency so the PE HAM clock
gate reaches 8/8 (2.4 GHz) before the first real matmul arrives.

PSUM->SBUF moves are split across Vector and GpSimd so neither engine
gates the PE pipeline and ScalarE stays op-free (no ACT table load).
"""

import numpy as np

N_CORES = 8
N_CLUSTERS = 100
P = 128
WARM_MMS = 18

_prog_cache: dict = {}


def _build_program(cpc: int, cap: int, in_c: int, f_out: int, fp8_path: bool):
    """Build + compile the per-core Bass program.

    cpc: clusters per core; cap: padded cluster size (multiple of 128).
    fp8_path: adjacency ships as raw fp8e4m3 counts (exact); the fp16
    fallback ships pre-scaled B^T blocks (counts > 16 only).
    """
    import concourse.mybir as mybir
    import concourse.tile as tile
    from concourse import bacc

    key = (cpc, cap, in_c, f_out, fp8_path)
    if key in _prog_cache:
        return _prog_cache[key]

    kc = in_c // P           # k chunks (step-1 stationary cols / step-2 contraction)
    sch = cap // P           # s chunks per cluster (step-1 contraction)
    fc = f_out // P          # f chunks (step-2 output partitions)
    f16 = mybir.dt.float16
    f32 = mybir.dt.float32
    a_dt = mybir.dt.float8e4 if fp8_path else f16

    XG = 4                   # clusters per X load group
    AG = 3                   # clusters per At load group

    nc = bacc.Bacc("TRN2", target_bir_lowering=False, debug=False,
                   num_devices=N_CORES)

    XS = nc.dram_tensor("XS", [P, cpc, sch, in_c], f16, kind="ExternalInput")
    WT = nc.dram_tensor("WT", [P, kc, fc, P], f16, kind="ExternalInput")
    AT = nc.dram_tensor("AT", [cpc, P, sch, cap], a_dt, kind="ExternalInput")
    YT = nc.dram_tensor("YT", [cpc, f_out, cap], f16, kind="ExternalOutput")

    with tile.TileContext(nc) as tc:
        with (
            tc.tile_pool(name="w", bufs=1) as w_pool,
            tc.tile_pool(name="jk", bufs=1) as jk_pool,
            tc.tile_pool(name="xt", bufs=3) as xt_pool,
            tc.tile_pool(name="at", bufs=3) as at_pool,
            tc.tile_pool(name="tsb", bufs=4) as tsb_pool,
            tc.tile_pool(name="out", bufs=4) as out_pool,
            tc.tile_pool(name="psw", bufs=1, space="PSUM") as psw_pool,
            tc.tile_pool(name="ps1", bufs=3, space="PSUM") as ps1_pool,
            tc.tile_pool(name="ps2", bufs=4, space="PSUM") as ps2_pool,
        ):
            # warm-up burst: junk matmuls with no input deps keep the PE
            # busy through the initial DMA latency and un-throttle HAM
            junk = jk_pool.tile([P, P], f16)
            jout = jk_pool.tile([P, 8], f16)
            nc.gpsimd.memset(junk[:], 0.0)
            psw = psw_pool.tile([P, P], f32)
            for i in range(WARM_MMS):
                nc.tensor.matmul(
                    psw[:], lhsT=junk[:], rhs=junk[:],
                    start=(i == 0), stop=(i == WARM_MMS - 1),
                )
            nc.vector.tensor_copy(jout[:], psw[:, :8])

            wt = w_pool.tile([P, kc, fc, P], f16)

            # small first groups so compute starts early
            def groups(g):
                sizes, c0 = [], 0
                first = True
                while c0 < cpc:
                    g_ = 1 if first else min(g, cpc - c0)
                    sizes.append((c0, g_))
                    c0 += g_
                    first = False
                return sizes

            xg_of = {}
            for c0, g in groups(XG):
                for c in range(c0, c0 + g):
                    xg_of[c] = (c0, g)
            ag_of = {}
            for c0, g in groups(AG):
                for c in range(c0, c0 + g):
                    ag_of[c] = (c0, g)

            xt = at = None
            for c in range(cpc):
                c0, g = xg_of[c]
                if c == c0:
                    xt = xt_pool.tile([P, XG, sch, in_c], f16)
                    nc.sync.dma_start(xt[:, :g], XS[:, c0:c0 + g])
                a0, ag = ag_of[c]
                if c == a0:
                    at = at_pool.tile([P, AG, sch, cap], a_dt)
                    nc.sync.dma_start(
                        at[:, :ag],
                        AT[a0:a0 + ag].rearrange("c p s d -> p c s d"),
                    )
                if c == 0:
                    # W after cluster 0's X/At: not needed until step 2
                    nc.sync.dma_start(wt[:], WT[:])
                xi = c - c0
                ci = c - a0

                # step 1: T^T = sum_s Xs-chunk^T x At rows, k on partitions
                tsb = tsb_pool.tile([P, kc, cap], f16)
                for k in range(kc):
                    ps = ps1_pool.tile([P, cap], f32)
                    for s in range(sch):
                        nc.tensor.matmul(
                            ps[:],
                            lhsT=xt[:, xi, s, k * P:(k + 1) * P],
                            rhs=at[:, ci, s, :],
                            start=(s == 0),
                            stop=(s == sch - 1),
                        )
                    if k == 0:
                        nc.vector.tensor_copy(tsb[:, k, :], ps[:])
                    else:
                        nc.scalar.copy(tsb[:, k, :], ps[:])

                # step 2: Y^T = sum_k W-chunk^T x T^T, f on partitions
                ot = out_pool.tile([P, fc, cap], f16)
                for f in range(fc):
                    ps = ps2_pool.tile([P, cap], f32)
                    for k in range(kc):
                        nc.tensor.matmul(
                            ps[:],
                            lhsT=wt[:, k, f, :],
                            rhs=tsb[:, k, :],
                            start=(k == 0),
                            stop=(k == kc - 1),
                        )
                    if f == 0:
                        nc.vector.tensor_copy(ot[:, f, :], ps[:])
                    else:
                        nc.scalar.copy(ot[:, f, :], ps[:])
                nc.sync.dma_start(
                    YT[c].rearrange("(f p) d -> p f d", p=P), ot[:]
                )

    nc.compile()
    _prog_cache[key] = nc
    return nc


def _host_prep(X, W, b, assign, full_ei):
    """Shard + preprocess. Returns (in_maps, fp8_path, gather info)."""
    n, in_c = X.shape
    f_out = W.shape[1]
    src = full_ei[0].astype(np.int64)
    dst = full_ei[1].astype(np.int64)
    a_s = assign[src]
    intra = a_s == assign[dst]
    es, ed = src[intra], dst[intra]

    deg = np.ones(n, np.float32)
    np.add.at(deg, ed, np.float32(1))
    dis = (1.0 / np.sqrt(deg)).astype(np.float32)

    has_edge = np.zeros(N_CLUSTERS, bool)
    has_edge[np.unique(a_s[intra])] = True

    sizes = np.bincount(assign, minlength=N_CLUSTERS)
    cpc = -(-N_CLUSTERS // N_CORES)            # clusters per core
    cap = max(512, int(-(-sizes.max() // P)) * P)  # padded cluster size
    kc = in_c // P
    fc = f_out // P
    sch = cap // P

    starts = np.zeros(N_CLUSTERS + 1, np.int64)
    starts[1:] = np.cumsum(sizes)
    order = np.argsort(assign, kind="stable")
    pos = np.empty(n, np.int64)
    pos[order] = np.arange(n) - starts[assign[order]]

    ctot = cpc * N_CORES
    # At blocks: At[c][s, d] = #edges(s->d) + [s==d]
    At = np.zeros((ctot, cap, cap), np.uint16)
    np.add.at(At, (assign[es], pos[es], pos[ed]), 1)
    At[assign, pos, pos] += 1
    fp8_path = int(At.max()) <= 16    # integers <= 16 are exact in e4m3

    Xp = np.zeros((ctot, cap, in_c), np.float32)
    if fp8_path:
        # fold dis[s] into the node features; host applies dis[d] at the end
        Xp[assign, pos] = X * dis[:, None]
        import concourse.mybir as mybir
        At_send = At.astype(mybir.dt.np(mybir.dt.float8e4))
    else:
        # rare fallback: fully pre-scaled B^T blocks in fp16
        Xp[assign, pos] = X
        DISp = np.zeros((ctot, cap), np.float32)
        DISp[assign, pos] = dis
        At_send = (At.astype(np.float32)
                   * DISp[:, :, None] * DISp[:, None, :]).astype(np.float16)
    # [c, s, d] -> [c, p, so, d]: partition p holds rows s = so*P + p
    At_send = np.ascontiguousarray(
        At_send.reshape(-1, sch, P, cap).transpose(0, 2, 1, 3))
    # X: [c, s, k] -> [p, c, so, k]
    XS_all = np.ascontiguousarray(
        Xp.reshape(ctot, sch, P, in_c).transpose(2, 0, 1, 3)
    ).astype(np.float16)
    WT_send = np.ascontiguousarray(
        W.astype(np.float32).reshape(kc, P, fc, P).transpose(1, 0, 2, 3)
    ).astype(np.float16)

    in_maps = []
    for i in range(N_CORES):
        in_maps.append({
            "XS": np.ascontiguousarray(XS_all[:, i * cpc:(i + 1) * cpc]),
            "WT": WT_send,
            "AT": At_send[i * cpc:(i + 1) * cpc],
        })
    return in_maps, fp8_path, (cpc, cap, has_edge, pos, dis)


def _run(inputs, trace=False, tmpdir=None):
    from concourse.bass_utils import run_bass_kernel_spmd

    X = np.asarray(inputs["X"], np.float32)
    W = np.asarray(inputs["W"], np.float32)
    b = np.asarray(inputs["b"], np.float32)
    assign = np.asarray(inputs["assign"])
    full_ei = np.asarray(inputs["full_ei"])

    n, in_c = X.shape
    f_out = W.shape[1]
    in_maps, fp8_path, (cpc, cap, has_edge, pos, dis) = _host_prep(
        X, W, b, assign, full_ei)
    nc = _build_program(cpc, cap, in_c, f_out, fp8_path)

    res = run_bass_kernel_spmd(
        nc, in_maps, core_ids=list(range(N_CORES)),
        trace=trace, tmpdir=tmpdir,
    )
    # YT: [core][cpc, f_out, cap]; row n lives at [core, lc, :, pos]
    YTdev = np.stack([res.results[i]["YT"] for i in range(N_CORES)])
    if YTdev.dtype != np.float32:
        YTdev = YTdev.astype(np.float32)

    c = assign.astype(np.int64)
    core = c // cpc
    lc = c % cpc
    Y = YTdev[core, lc, :, pos]
    if fp8_path:
        Y *= dis[:, None]
    Y += b[None, :].astype(np.float32)
    miss = ~has_edge[c]
    if miss.any():
        Y[miss] = X[miss]
    return Y, res


def kernel(**inputs) -> np.ndarray:
    Y, _ = _run(inputs)
    return Y
